# revision 10
# baseline (speedup 1.0000x reference)
"""ConvS2S-style attention (nn_Attention_3332894621985) on 8 Trainium2 NeuronCores.

Data-parallel over batch B=8: core i computes batch element i end to end.

Per-core math (batch b), shapes hardcoded for B,C,T,S,E = 8,1024,2048,2048,1024:
    cc     = s*W_h2e @ conved[b] + s*(x[b].T + b_h2e)   : [E, T]   (MM1)
    energy = cc.T @ encT                                : [T, S]   (MM2)
    att    = softmax(energy, axis=-1)  -> output 1 (fp32)
    att_T  = PE-transpose(att)                          : [S, T]
    attd_T = encC.T @ att_T                             : [E, T]   (MM3)
    out    = W_e2h @ attd + conved[b] + b_e2h           : [C, T]   (MM4) -> output 2

Precision: MM1 and MM2 run as 3-term fp16 products (hi/lo split of both
operands, fp32 PSUM accumulation) because softmax over S=2048 with E=1024
contractions amplifies operand rounding ~sqrt(E); single-rounded operands
give ~6e-2 (bf16) / ~7e-3 (fp16) attention error, while the 3-term scheme
is ~1e-5.  MM3/MM4 operands are single-rounded fp16 (fp16 and bf16 both run
the PE at 1 cycle/row; fp16 has 8x finer mantissa).
"""
import numpy as np
import ml_dtypes

F16 = np.float16

B, C, T, S, E = 8, 1024, 2048, 2048, 1024
P = 128
EC, KC, SC, TC = E // P, C // P, S // P, T // P   # 8, 8, 16, 16
NT = T // 512                                     # t-slices of 512

_CACHE = {}


def _split_f16(a):
    hi = a.astype(F16)
    lo = (a - hi.astype(np.float32)).astype(F16)
    return hi, lo


def _build():
    import concourse.bacc as bacc
    import concourse.tile as tile
    from concourse import mybir
    from concourse.masks import make_identity

    F32 = mybir.dt.float32
    BF = mybir.dt.float16

    nc = bacc.Bacc("TRN2", target_bir_lowering=False, debug=False)

    def din(name, shape, dt=BF):
        return nc.dram_tensor(name, shape, dt, kind="ExternalInput").ap()

    w1h_d = din("w1h", [C, E])
    w1l_d = din("w1l", [C, E])
    cvh_d = din("cvh", [C, T])
    cvl_d = din("cvl", [C, T])
    cres_d = din("cres", [C, T], F32)
    xs_d = din("xs", [E, T], F32)
    ekh_d = din("ekh", [E, S])
    ekl_d = din("ekl", [E, S])
    ev_d = din("ev", [S, E])
    w2t_d = din("w2t", [E, C])

    att_d = nc.dram_tensor("attention", [T, S], F32, kind="ExternalOutput").ap()
    out_d = nc.dram_tensor("out", [C, T], F32, kind="ExternalOutput").ap()

    # cc spill scratch, packed partition-major [p, e-chunk, t]: both the
    # phase-1 write (fixed e, t-slice) and the phase-2 read (fixed t-chunk,
    # all e-chunks) are single strided DMAs.
    cch_d = nc.dram_tensor("cch", [P, EC * T], BF, kind="Internal").ap()
    ccl_d = nc.dram_tensor("ccl", [P, EC * T], BF, kind="Internal").ap()
    abf_d = nc.dram_tensor("abf", [TC * P, S], BF, kind="Internal").ap()

    w1h_r = w1h_d.rearrange("(k p) e -> k p e", p=P)
    w1l_r = w1l_d.rearrange("(k p) e -> k p e", p=P)
    cvh_r = cvh_d.rearrange("(k p) t -> k p t", p=P)
    cvl_r = cvl_d.rearrange("(k p) t -> k p t", p=P)
    cres_r = cres_d.rearrange("(c p) t -> c p t", p=P)
    xs_r = xs_d.rearrange("(e p) t -> e p t", p=P)
    ekh_r = ekh_d.rearrange("(e p) s -> e p s", p=P)
    ekl_r = ekl_d.rearrange("(e p) s -> e p s", p=P)
    ev_r = ev_d.rearrange("(s p) e -> s p e", p=P)
    w2t_r = w2t_d.rearrange("(e p) c -> e p c", p=P)
    att_r = att_d.rearrange("(t p) s -> t p s", p=P)
    out_r = out_d.rearrange("(c p) t -> c p t", p=P)
    cch_r = cch_d.rearrange("p (e t) -> p e t", e=EC)
    ccl_r = ccl_d.rearrange("p (e t) -> p e t", e=EC)
    abf_r = abf_d.rearrange("(t p) s -> t p s", p=P)

    with tile.TileContext(nc) as tc:
        # Pool lifetimes (strict LIFO): ident/ev/w2 span everything (their
        # loads overlap phase 1); ek spans phases 1-2 and closes before the
        # phase-3 pools open so everything fits in ~208 KB/partition.
        with tc.tile_pool(name="evw", bufs=1) as abp:
            ev_sb = abp.tile([P, SC, E], BF)
            w2_sb = abp.tile([P, EC, C], BF)

            with tc.tile_pool(name="ek", bufs=1) as ekp:
                ekh_sb = ekp.tile([P, EC, S], BF)
                ekl_sb = ekp.tile([P, EC, S], BF)

                # ---------- Phase 1: MM1, cc hi/lo spilled to DRAM ----------
                with tc.tile_pool(name="w1", bufs=1) as w1p, \
                     tc.tile_pool(name="cvs", bufs=2) as cvsp, \
                     tc.tile_pool(name="xs", bufs=1) as xsp, \
                     tc.tile_pool(name="cc", bufs=2) as ccp, \
                     tc.tile_pool(name="ps1", bufs=2, space="PSUM") as pp1:
                    w1h_sb = w1p.tile([P, KC, E], BF)
                    w1l_sb = w1p.tile([P, KC, E], BF)
                    for k in range(KC):
                        nc.sync.dma_start(w1h_sb[:, k, :], w1h_r[k])
                        nc.sync.dma_start(w1l_sb[:, k, :], w1l_r[k])

                    def load_slice(ts):
                        tsl = slice(ts * 512, (ts + 1) * 512)
                        cvh_t = cvsp.tile([P, KC, 512], BF, tag="cvh",
                                          name=f"cvh_{ts}")
                        cvl_t = cvsp.tile([P, KC, 512], BF, tag="cvl",
                                          name=f"cvl_{ts}")
                        xs_t = xsp.tile([P, EC, 512], F32, tag="xs",
                                        name=f"xs_{ts}")
                        for k in range(KC):
                            nc.sync.dma_start(cvh_t[:, k, :], cvh_r[k][:, tsl])
                            nc.sync.dma_start(cvl_t[:, k, :], cvl_r[k][:, tsl])
                        for e in range(EC):
                            nc.sync.dma_start(xs_t[:, e, :], xs_r[e][:, tsl])
                        return cvh_t, cvl_t, xs_t

                    slice0 = load_slice(0)
                    # prefetches for later phases: issued after slice-0's loads
                    # (SWDGE rings, so they don't block later cv slices on the
                    # HWDGE queues either)
                    for e in range(EC):
                        nc.gpsimd.dma_start(ekh_sb[:, e, :], ekh_r[e])
                        nc.gpsimd.dma_start(ekl_sb[:, e, :], ekl_r[e])
                    for s in range(SC):
                        nc.gpsimd.dma_start(ev_sb[:, s, :], ev_r[s])
                    for e in range(EC):
                        nc.gpsimd.dma_start(w2_sb[:, e, :], w2t_r[e])

                    for ts in range(NT):
                        tsl = slice(ts * 512, (ts + 1) * 512)
                        cvh_t, cvl_t, xs_t = slice0 if ts == 0 else load_slice(ts)
                        for e in range(EC):
                            ps1 = pp1.tile([P, 512], F32, tag="mm1")
                            for k in range(KC):
                                terms = [(w1h_sb, cvh_t), (w1h_sb, cvl_t),
                                         (w1l_sb, cvh_t)]
                                for pi, (wt, cv) in enumerate(terms):
                                    nc.tensor.matmul(
                                        ps1[:], wt[:, k, e * P:(e + 1) * P],
                                        cv[:, k, :],
                                        start=(k == 0 and pi == 0),
                                        stop=(k == KC - 1 and pi == 2))
                            cc32 = ccp.tile([P, 512], F32, tag="cc32")
                            nc.vector.tensor_add(cc32[:], ps1[:], xs_t[:, e, :])
                            cch_sb = ccp.tile([P, 512], BF, tag="cch")
                            nc.scalar.copy(cch_sb[:], cc32[:])
                            ccl_sb = ccp.tile([P, 512], BF, tag="ccl")
                            nc.vector.tensor_tensor(
                                out=ccl_sb[:], in0=cc32[:], in1=cch_sb[:],
                                op=mybir.AluOpType.subtract)
                            nc.sync.dma_start(cch_r[:, e, tsl], cch_sb[:])
                            nc.sync.dma_start(ccl_r[:, e, tsl], ccl_sb[:])

                # ---------- Phase 2: energy + softmax -----------------------
                with tc.tile_pool(name="ccs", bufs=2) as ccsp, \
                     tc.tile_pool(name="att", bufs=2) as attp, \
                     tc.tile_pool(name="st", bufs=2) as stp, \
                     tc.tile_pool(name="ps2", bufs=2, space="PSUM") as pp2:
                    for tci in range(TC):
                        cct_h = ccsp.tile([P, EC, P], BF, tag="ccth")
                        cct_l = ccsp.tile([P, EC, P], BF, tag="cctl")
                        nc.sync.dma_start(cct_h[:], cch_r[:, :, tci * P:(tci + 1) * P])
                        nc.sync.dma_start(cct_l[:], ccl_r[:, :, tci * P:(tci + 1) * P])
                        eps = pp2.tile([P, S], F32, tag="energy")
                        for k in range(EC):
                            terms = [(cct_h, ekh_sb), (cct_h, ekl_sb),
                                     (cct_l, ekh_sb)]
                            for pi, (l, r) in enumerate(terms):
                                for ss in range(4):
                                    nc.tensor.matmul(
                                        eps[:, ss * 512:(ss + 1) * 512],
                                        l[:, k, :], r[:, k, ss * 512:(ss + 1) * 512],
                                        start=(k == 0 and pi == 0),
                                        stop=(k == EC - 1 and pi == 2))
                        negmax = stp.tile([P, 1], F32, tag="nm")
                        nc.vector.tensor_reduce(out=negmax[:], in_=eps[:],
                                                op=mybir.AluOpType.max,
                                                axis=mybir.AxisListType.X,
                                                negate=True)
                        attf = attp.tile([P, S], F32, tag="attf")
                        ssum = stp.tile([P, 1], F32, tag="sum")
                        nc.scalar.activation(out=attf[:], in_=eps[:],
                                             func=mybir.ActivationFunctionType.Exp,
                                             bias=negmax[:], scale=1.0,
                                             accum_out=ssum[:])
                        rcp = stp.tile([P, 1], F32, tag="rcp")
                        nc.vector.reciprocal(rcp[:], ssum[:])
                        nc.vector.tensor_scalar_mul(attf[:], in0=attf[:],
                                                    scalar1=rcp[:])
                        nc.sync.dma_start(att_r[tci], attf[:])
                        abf_o = attp.tile([P, S], BF, tag="abfo")
                        nc.scalar.copy(abf_o[:], attf[:])
                        nc.sync.dma_start(abf_r[tci], abf_o[:])

            # ---------- Phases 3+4: transpose, MM3, MM4, residual -----------
            with tc.tile_pool(name="aT", bufs=2) as aTp, \
                 tc.tile_pool(name="attT", bufs=2) as attTp, \
                 tc.tile_pool(name="res", bufs=2) as resp, \
                 tc.tile_pool(name="osb", bufs=2) as outp, \
                 tc.tile_pool(name="ps3", bufs=2, space="PSUM") as pp3:
                for ts in range(NT):
                    # aT[sb] = att[ts*512:(ts+1)*512, sb*128:(sb+1)*128].T via
                    # the DMA xbar (2-byte transpose), straight from the abf
                    # DRAM spill -- no PE/DVE involvement.
                    aT = aTp.tile([P, SC, 512], BF, tag="aT")
                    for sb in range(SC):
                        nc.sync.dma_start_transpose(
                            out=aT[:, sb, :],
                            in_=abf_d[ts * 512:(ts + 1) * 512,
                                      sb * P:(sb + 1) * P])
                    attT = attTp.tile([P, EC, 512], BF, tag="attT")
                    for e in range(EC):
                        ps3 = pp3.tile([P, 512], F32, tag="mm3")
                        for sb in range(SC):
                            nc.tensor.matmul(ps3[:], ev_sb[:, sb, e * P:(e + 1) * P],
                                             aT[:, sb, :],
                                             start=(sb == 0), stop=(sb == SC - 1))
                        nc.vector.tensor_copy(attT[:, e, :], ps3[:])
                    res = resp.tile([P, KC, 512], F32, tag="res")
                    for c in range(KC):
                        nc.sync.dma_start(res[:, c, :],
                                          cres_r[c][:, ts * 512:(ts + 1) * 512])
                    for c in range(KC):
                        ps4 = pp3.tile([P, 512], F32, tag="mm4")
                        for e in range(EC):
                            nc.tensor.matmul(ps4[:], w2_sb[:, e, c * P:(c + 1) * P],
                                             attT[:, e, :],
                                             start=(e == 0), stop=(e == EC - 1))
                        osb = outp.tile([P, 512], F32, tag="osb")
                        nc.vector.tensor_add(osb[:], ps4[:], res[:, c, :])
                        nc.sync.dma_start(out_r[c][:, ts * 512:(ts + 1) * 512],
                                          osb[:])
    nc.compile()
    return nc


def kernel(conved, encoder_conved, encoder_combined, x, scale,
           W_h2e, b_h2e, W_e2h, b_e2h):
    import concourse.bass_utils as bass_utils

    conved = np.asarray(conved, dtype=np.float32)
    encoder_conved = np.asarray(encoder_conved, dtype=np.float32)
    encoder_combined = np.asarray(encoder_combined, dtype=np.float32)
    x = np.asarray(x, dtype=np.float32)
    W_h2e = np.asarray(W_h2e, dtype=np.float32)
    b_h2e = np.asarray(b_h2e, dtype=np.float32)
    W_e2h = np.asarray(W_e2h, dtype=np.float32)
    b_e2h = np.asarray(b_e2h, dtype=np.float32)
    s = np.float32(scale)

    if "nc" not in _CACHE:
        _CACHE["nc"] = _build()
    nc = _CACHE["nc"]

    w1h, w1l = _split_f16(np.ascontiguousarray((s * W_h2e).T))      # [C, E]
    w2t = np.ascontiguousarray(W_e2h.T).astype(F16)                 # [E, C]

    in_maps = []
    for b in range(B):
        cv = conved[b]                                               # [C, T]
        cvh, cvl = _split_f16(cv)
        cres = cv + b_e2h[:, None]                                   # [C, T]
        xs = np.ascontiguousarray((s * (x[b] + b_h2e[None, :])).T)   # [E, T]
        ekh, ekl = _split_f16(np.ascontiguousarray(encoder_conved[b].T))  # [E, S]
        ev = encoder_combined[b].astype(F16)                        # [S, E]
        in_maps.append({
            "w1h": w1h, "w1l": w1l,
            "cvh": np.ascontiguousarray(cvh), "cvl": np.ascontiguousarray(cvl),
            "cres": np.ascontiguousarray(cres), "xs": xs,
            "ekh": np.ascontiguousarray(ekh), "ekl": np.ascontiguousarray(ekl),
            "ev": np.ascontiguousarray(ev), "w2t": w2t,
        })

    res = bass_utils.run_bass_kernel_spmd(nc, in_maps, core_ids=list(range(B)))
    attention = np.stack([res.results[b]["attention"] for b in range(B)])
    attended_combined = np.stack([res.results[b]["out"] for b in range(B)])
    return attention.astype(np.float32), attended_combined.astype(np.float32)


# revision 11
# speedup vs baseline: 1.0858x; 1.0858x over previous
"""ConvS2S-style attention (nn_Attention_3332894621985) on 8 Trainium2 NeuronCores.

Data-parallel over batch B=8: core i computes batch element i end to end.

Per-core math (batch b), shapes hardcoded for B,C,T,S,E = 8,1024,2048,2048,1024:
    cc     = s*W_h2e @ conved[b] + s*(x[b].T + b_h2e)   : [E, T]   (MM1)
    energy = cc.T @ encT                                : [T, S]   (MM2)
    att    = softmax(energy, axis=-1)  -> output 1 (fp32)
    att_T  = PE-transpose(att)                          : [S, T]
    attd_T = encC.T @ att_T                             : [E, T]   (MM3)
    out    = W_e2h @ attd + conved[b] + b_e2h           : [C, T]   (MM4) -> output 2

Precision: MM1 and MM2 run as 3-term fp16 products (hi/lo split of both
operands, fp32 PSUM accumulation) because softmax over S=2048 with E=1024
contractions amplifies operand rounding ~sqrt(E); single-rounded operands
give ~6e-2 (bf16) / ~7e-3 (fp16) attention error, while the 3-term scheme
is ~1e-5.  MM3/MM4 operands are single-rounded fp16 (fp16 and bf16 both run
the PE at 1 cycle/row; fp16 has 8x finer mantissa).
"""
import numpy as np
import ml_dtypes

F16 = np.float16

B, C, T, S, E = 8, 1024, 2048, 2048, 1024
P = 128
EC, KC, SC, TC = E // P, C // P, S // P, T // P   # 8, 8, 16, 16
NT = T // 512                                     # t-slices of 512

_CACHE = {}


def _split_f16(a):
    hi = a.astype(F16)
    lo = (a - hi.astype(np.float32)).astype(F16)
    return hi, lo


def _build():
    import concourse.bacc as bacc
    import concourse.tile as tile
    from concourse import mybir
    from concourse.masks import make_identity

    F32 = mybir.dt.float32
    BF = mybir.dt.float16

    nc = bacc.Bacc("TRN2", target_bir_lowering=False, debug=False)

    def din(name, shape, dt=BF):
        return nc.dram_tensor(name, shape, dt, kind="ExternalInput").ap()

    w1h_d = din("w1h", [C, E])
    w1l_d = din("w1l", [C, E])
    cvh_d = din("cvh", [C, T])
    cvl_d = din("cvl", [C, T])
    cres_d = din("cres", [C, T], F32)
    xs_d = din("xs", [E, T], F32)
    ekh_d = din("ekh", [E, S])
    ekl_d = din("ekl", [E, S])
    ev_d = din("ev", [S, E])
    w2t_d = din("w2t", [E, C])

    att_d = nc.dram_tensor("attention", [T, S], F32, kind="ExternalOutput").ap()
    out_d = nc.dram_tensor("out", [C, T], F32, kind="ExternalOutput").ap()

    # cc spill scratch, packed partition-major [p, e-chunk, t]: both the
    # phase-1 write (fixed e, t-slice) and the phase-2 read (fixed t-chunk,
    # all e-chunks) are single strided DMAs.
    cch_d = nc.dram_tensor("cch", [P, EC * T], BF, kind="Internal").ap()
    ccl_d = nc.dram_tensor("ccl", [P, EC * T], BF, kind="Internal").ap()
    abf_d = nc.dram_tensor("abf", [TC * P, S], BF, kind="Internal").ap()

    w1h_r = w1h_d.rearrange("(k p) e -> k p e", p=P)
    w1l_r = w1l_d.rearrange("(k p) e -> k p e", p=P)
    cvh_r = cvh_d.rearrange("(k p) t -> k p t", p=P)
    cvl_r = cvl_d.rearrange("(k p) t -> k p t", p=P)
    cres_r = cres_d.rearrange("(c p) t -> c p t", p=P)
    xs_r = xs_d.rearrange("(e p) t -> e p t", p=P)
    ekh_r = ekh_d.rearrange("(e p) s -> e p s", p=P)
    ekl_r = ekl_d.rearrange("(e p) s -> e p s", p=P)
    ev_r = ev_d.rearrange("(s p) e -> s p e", p=P)
    w2t_r = w2t_d.rearrange("(e p) c -> e p c", p=P)
    att_r = att_d.rearrange("(t p) s -> t p s", p=P)
    out_r = out_d.rearrange("(c p) t -> c p t", p=P)
    cch_r = cch_d.rearrange("p (e t) -> p e t", e=EC)
    ccl_r = ccl_d.rearrange("p (e t) -> p e t", e=EC)
    abf_r = abf_d.rearrange("(t p) s -> t p s", p=P)

    with tile.TileContext(nc) as tc:
        # Pool lifetimes (strict LIFO): ident/ev/w2 span everything (their
        # loads overlap phase 1); ek spans phases 1-2 and closes before the
        # phase-3 pools open so everything fits in ~208 KB/partition.
        with tc.tile_pool(name="evw", bufs=1) as abp:
            ev_sb = abp.tile([P, SC, E], BF)
            w2_sb = abp.tile([P, EC, C], BF)

            with tc.tile_pool(name="ek", bufs=1) as ekp:
                ekh_sb = ekp.tile([P, EC, S], BF)
                ekl_sb = ekp.tile([P, EC, S], BF)

                # ---------- Phase 1: MM1, cc hi/lo spilled to DRAM ----------
                with tc.tile_pool(name="w1", bufs=1) as w1p, \
                     tc.tile_pool(name="cvs", bufs=2) as cvsp, \
                     tc.tile_pool(name="xs", bufs=1) as xsp, \
                     tc.tile_pool(name="cc", bufs=2) as ccp, \
                     tc.tile_pool(name="ps1", bufs=2, space="PSUM") as pp1:
                    w1h_sb = w1p.tile([P, KC, E], BF)
                    w1l_sb = w1p.tile([P, KC, E], BF)
                    for k in range(KC):
                        nc.sync.dma_start(w1h_sb[:, k, :], w1h_r[k])
                        nc.sync.dma_start(w1l_sb[:, k, :], w1l_r[k])

                    def load_slice(ts):
                        tsl = slice(ts * 512, (ts + 1) * 512)
                        cvh_t = cvsp.tile([P, KC, 512], BF, tag="cvh",
                                          name=f"cvh_{ts}")
                        cvl_t = cvsp.tile([P, KC, 512], BF, tag="cvl",
                                          name=f"cvl_{ts}")
                        xs_t = xsp.tile([P, EC, 512], F32, tag="xs",
                                        name=f"xs_{ts}")
                        for k in range(KC):
                            nc.sync.dma_start(cvh_t[:, k, :], cvh_r[k][:, tsl])
                            nc.sync.dma_start(cvl_t[:, k, :], cvl_r[k][:, tsl])
                        for e in range(EC):
                            nc.sync.dma_start(xs_t[:, e, :], xs_r[e][:, tsl])
                        return cvh_t, cvl_t, xs_t

                    slice0 = load_slice(0)
                    # Prefetches for later phases, issued AFTER slice-0's loads
                    # on the same HWDGE queues: per-queue FIFO order guarantees
                    # slice 0 wins the HBM race, while the prefetches still
                    # overlap the rest of phase-1 compute.
                    for e in range(EC):
                        nc.sync.dma_start(ekh_sb[:, e, :], ekh_r[e])
                        nc.sync.dma_start(ekl_sb[:, e, :], ekl_r[e])
                    for s in range(SC):
                        nc.sync.dma_start(ev_sb[:, s, :], ev_r[s])
                    for e in range(EC):
                        nc.sync.dma_start(w2_sb[:, e, :], w2t_r[e])

                    for ts in range(NT):
                        tsl = slice(ts * 512, (ts + 1) * 512)
                        cvh_t, cvl_t, xs_t = slice0 if ts == 0 else load_slice(ts)
                        for e in range(EC):
                            ps1 = pp1.tile([P, 512], F32, tag="mm1")
                            for k in range(KC):
                                terms = [(w1h_sb, cvh_t), (w1h_sb, cvl_t),
                                         (w1l_sb, cvh_t)]
                                for pi, (wt, cv) in enumerate(terms):
                                    nc.tensor.matmul(
                                        ps1[:], wt[:, k, e * P:(e + 1) * P],
                                        cv[:, k, :],
                                        start=(k == 0 and pi == 0),
                                        stop=(k == KC - 1 and pi == 2))
                            cc32 = ccp.tile([P, 512], F32, tag="cc32")
                            nc.vector.tensor_add(cc32[:], ps1[:], xs_t[:, e, :])
                            cch_sb = ccp.tile([P, 512], BF, tag="cch")
                            nc.scalar.copy(cch_sb[:], cc32[:])
                            ccl_sb = ccp.tile([P, 512], BF, tag="ccl")
                            nc.vector.tensor_tensor(
                                out=ccl_sb[:], in0=cc32[:], in1=cch_sb[:],
                                op=mybir.AluOpType.subtract)
                            nc.sync.dma_start(cch_r[:, e, tsl], cch_sb[:])
                            nc.sync.dma_start(ccl_r[:, e, tsl], ccl_sb[:])

                # ---------- Phase 2: energy + softmax -----------------------
                with tc.tile_pool(name="ccs", bufs=2) as ccsp, \
                     tc.tile_pool(name="att", bufs=2) as attp, \
                     tc.tile_pool(name="st", bufs=2) as stp, \
                     tc.tile_pool(name="ps2", bufs=2, space="PSUM") as pp2:
                    for tci in range(TC):
                        cct_h = ccsp.tile([P, EC, P], BF, tag="ccth")
                        cct_l = ccsp.tile([P, EC, P], BF, tag="cctl")
                        nc.sync.dma_start(cct_h[:], cch_r[:, :, tci * P:(tci + 1) * P])
                        nc.sync.dma_start(cct_l[:], ccl_r[:, :, tci * P:(tci + 1) * P])
                        eps = pp2.tile([P, S], F32, tag="energy")
                        for k in range(EC):
                            terms = [(cct_h, ekh_sb), (cct_h, ekl_sb),
                                     (cct_l, ekh_sb)]
                            for pi, (l, r) in enumerate(terms):
                                for ss in range(4):
                                    nc.tensor.matmul(
                                        eps[:, ss * 512:(ss + 1) * 512],
                                        l[:, k, :], r[:, k, ss * 512:(ss + 1) * 512],
                                        start=(k == 0 and pi == 0),
                                        stop=(k == EC - 1 and pi == 2))
                        negmax = stp.tile([P, 1], F32, tag="nm")
                        nc.vector.tensor_reduce(out=negmax[:], in_=eps[:],
                                                op=mybir.AluOpType.max,
                                                axis=mybir.AxisListType.X,
                                                negate=True)
                        attf = attp.tile([P, S], F32, tag="attf")
                        ssum = stp.tile([P, 1], F32, tag="sum")
                        nc.scalar.activation(out=attf[:], in_=eps[:],
                                             func=mybir.ActivationFunctionType.Exp,
                                             bias=negmax[:], scale=1.0,
                                             accum_out=ssum[:])
                        rcp = stp.tile([P, 1], F32, tag="rcp")
                        nc.vector.reciprocal(rcp[:], ssum[:])
                        nc.vector.tensor_scalar_mul(attf[:], in0=attf[:],
                                                    scalar1=rcp[:])
                        nc.sync.dma_start(att_r[tci], attf[:])
                        abf_o = attp.tile([P, S], BF, tag="abfo")
                        nc.scalar.copy(abf_o[:], attf[:])
                        nc.sync.dma_start(abf_r[tci], abf_o[:])

            # ---------- Phases 3+4: transpose, MM3, MM4, residual -----------
            with tc.tile_pool(name="aT", bufs=1) as aTp, \
                 tc.tile_pool(name="attT", bufs=2) as attTp, \
                 tc.tile_pool(name="res", bufs=4) as resp, \
                 tc.tile_pool(name="osb", bufs=2) as outp, \
                 tc.tile_pool(name="ps3", bufs=2, space="PSUM") as pp3:
                # All 64 attention transposes via the DMA xbar (2-byte
                # transpose) straight from the abf DRAM spill -- one
                # contiguous xbar-mode region, so Tile's DMATranspose <->
                # DMACopy serialization fires only twice, and no PE/DVE
                # cycles are spent on transposes at all.
                aT = aTp.tile([P, SC, T], BF)       # 64 KB/partition
                for ts in range(NT):
                    for sb in range(SC):
                        nc.sync.dma_start_transpose(
                            out=aT[:, sb, ts * 512:(ts + 1) * 512],
                            in_=abf_d[ts * 512:(ts + 1) * 512,
                                      sb * P:(sb + 1) * P])
                reses = []
                for ts in range(NT):
                    res = resp.tile([P, KC, 512], F32, tag="res",
                                    name=f"res_{ts}")
                    for c in range(KC):
                        nc.sync.dma_start(res[:, c, :],
                                          cres_r[c][:, ts * 512:(ts + 1) * 512])
                    reses.append(res)
                for ts in range(NT):
                    res = reses[ts]
                    attT = attTp.tile([P, EC, 512], BF, tag="attT",
                                      name=f"attT_{ts}")
                    for e in range(EC):
                        ps3 = pp3.tile([P, 512], F32, tag="mm3",
                                       name=f"ps3_{ts}_{e}")
                        for sb in range(SC):
                            nc.tensor.matmul(ps3[:],
                                             ev_sb[:, sb, e * P:(e + 1) * P],
                                             aT[:, sb, ts * 512:(ts + 1) * 512],
                                             start=(sb == 0), stop=(sb == SC - 1))
                        nc.vector.tensor_copy(attT[:, e, :], ps3[:])
                    for c in range(KC):
                        ps4 = pp3.tile([P, 512], F32, tag="mm4",
                                       name=f"ps4_{ts}_{c}")
                        for e in range(EC):
                            nc.tensor.matmul(ps4[:], w2_sb[:, e, c * P:(c + 1) * P],
                                             attT[:, e, :],
                                             start=(e == 0), stop=(e == EC - 1))
                        osb = outp.tile([P, 512], F32, tag="osb")
                        nc.vector.tensor_add(osb[:], ps4[:], res[:, c, :])
                        nc.sync.dma_start(out_r[c][:, ts * 512:(ts + 1) * 512],
                                          osb[:])
    nc.compile()
    return nc


def kernel(conved, encoder_conved, encoder_combined, x, scale,
           W_h2e, b_h2e, W_e2h, b_e2h):
    import concourse.bass_utils as bass_utils

    conved = np.asarray(conved, dtype=np.float32)
    encoder_conved = np.asarray(encoder_conved, dtype=np.float32)
    encoder_combined = np.asarray(encoder_combined, dtype=np.float32)
    x = np.asarray(x, dtype=np.float32)
    W_h2e = np.asarray(W_h2e, dtype=np.float32)
    b_h2e = np.asarray(b_h2e, dtype=np.float32)
    W_e2h = np.asarray(W_e2h, dtype=np.float32)
    b_e2h = np.asarray(b_e2h, dtype=np.float32)
    s = np.float32(scale)

    if "nc" not in _CACHE:
        _CACHE["nc"] = _build()
    nc = _CACHE["nc"]

    w1h, w1l = _split_f16(np.ascontiguousarray((s * W_h2e).T))      # [C, E]
    w2t = np.ascontiguousarray(W_e2h.T).astype(F16)                 # [E, C]

    in_maps = []
    for b in range(B):
        cv = conved[b]                                               # [C, T]
        cvh, cvl = _split_f16(cv)
        cres = cv + b_e2h[:, None]                                   # [C, T]
        xs = np.ascontiguousarray((s * (x[b] + b_h2e[None, :])).T)   # [E, T]
        ekh, ekl = _split_f16(np.ascontiguousarray(encoder_conved[b].T))  # [E, S]
        ev = encoder_combined[b].astype(F16)                        # [S, E]
        in_maps.append({
            "w1h": w1h, "w1l": w1l,
            "cvh": np.ascontiguousarray(cvh), "cvl": np.ascontiguousarray(cvl),
            "cres": np.ascontiguousarray(cres), "xs": xs,
            "ekh": np.ascontiguousarray(ekh), "ekl": np.ascontiguousarray(ekl),
            "ev": np.ascontiguousarray(ev), "w2t": w2t,
        })

    res = bass_utils.run_bass_kernel_spmd(nc, in_maps, core_ids=list(range(B)))
    attention = np.stack([res.results[b]["attention"] for b in range(B)])
    attended_combined = np.stack([res.results[b]["out"] for b in range(B)])
    return attention.astype(np.float32), attended_combined.astype(np.float32)
